# revision 1
# baseline (speedup 1.0000x reference)
"""DisentangleLossBatch Trainium2 kernel (8 NeuronCores, data-parallel).

Math: loss = sum|mean_b(G[idx_g(b), idx_h(b)]) - I| over the 8x8 top-k
Gram matrix, where G = Cn @ Cn.T is the normalized-codebook Gram ([512,512])
and idx = top-8 indices of each token's 512 pose logits.

Key facts used:
  * inner[b,g,h] = Cn[i_g]·Cn[i_h] = G[i_g, i_h]  -> gather 28 (g<h) Gram
    entries per token instead of 8x256 codebook rows.
  * G[i,i] == 1 (normalized rows), so the diagonal of |mean - I| is ~0 and
    the loss is 2 * sum_{g<h} |mean[g,h]|.
  * top-8 == vector-engine max8/max_index instructions.
  * the gather is indirect_dma_start (software-DGE) with cce_op=add, which
    accumulates gathered entries straight into a [128, TB, 28] SBUF
    accumulator -- no on-chip select/mask work at all.

Per core (4096 tokens): load pose tiles [128,512]; max8 + max_index;
build 28 pair indices pidx = 512*i_g + i_h; gather-accumulate from G in
HBM; partition-reduce via ones-matmul; AllReduce [1,28] over 8 cores;
loss = (2/BN) * sum|entries|.
"""
import sys
import numpy as np

for _p in ("/opt/trn_rl_repo",):
    if _p not in sys.path:
        sys.path.insert(0, _p)

from contextlib import ExitStack

import concourse.bass as bass
import concourse.bacc as bacc
import concourse.tile as tile
import concourse.mybir as mybir
from concourse.bass import IndirectOffsetOnAxis
from concourse.bass_utils import run_bass_kernel_spmd

P = 128
N_CORES = 8
B, N, D, E = 32, 1024, 512, 256
G8 = 8
BN = B * N                       # 32768 tokens
BN_PER_CORE = BN // N_CORES      # 4096
T = BN_PER_CORE // P             # 32 tiles per core
TB = 32                          # tiles per gather batch
NB = T // TB                     # 8 batches
NPAIR = (G8 * (G8 - 1)) // 2     # 28 strictly-upper pairs
f32 = mybir.dt.float32
i32 = mybir.dt.int32
u32 = mybir.dt.uint32

# slot offsets for pair construction: pairs (g, h) with h > g
_PAIR_OFF = []
_off = 0
for _g in range(G8 - 1):
    _PAIR_OFF.append(_off)
    _off += G8 - 1 - _g
assert _off == NPAIR


def build_nc(debug=False):
    nc = bacc.Bacc("TRN2", target_bir_lowering=False, debug=False,
                   num_devices=N_CORES)
    pose = nc.dram_tensor("pose", [BN_PER_CORE, D], f32, kind="ExternalInput")
    cb = nc.dram_tensor("codebook", [D, E], f32, kind="ExternalInput")
    ident = nc.dram_tensor("ident", [P, P], f32, kind="ExternalInput")
    ones = nc.dram_tensor("ones", [P, 1], f32, kind="ExternalInput")
    loss = nc.dram_tensor("loss", [1, 1], f32, kind="ExternalOutput")
    g_hbm = nc.dram_tensor("g_scratch", [D * D + D], f32)
    ar_in = nc.dram_tensor("ar_in", [1, NPAIR], f32)
    if debug:
        d_idx = nc.dram_tensor("d_idx", [P, T * G8], u32, kind="ExternalOutput")
        d_acc = nc.dram_tensor("d_acc", [P, TB * NPAIR], f32, kind="ExternalOutput")
        d_red = nc.dram_tensor("d_red", [P, NPAIR], f32, kind="ExternalOutput")
        d_part = nc.dram_tensor("d_part", [1, NPAIR], f32, kind="ExternalOutput")
        d_allr = nc.dram_tensor("d_allr", [1, NPAIR], f32, kind="ExternalOutput")
        d_g = nc.dram_tensor("d_g", [P, 4 * D], f32, kind="ExternalOutput")
        d_gath = nc.dram_tensor("d_gath", [P, TB * NPAIR], f32, kind="ExternalOutput")
        d_pidx = nc.dram_tensor("d_pidx", [P, TB * NPAIR], i32, kind="ExternalOutput")
    ar_out = nc.dram_tensor("ar_out", [1, NPAIR], f32, addr_space="Shared")

    with tile.TileContext(nc) as tc, ExitStack() as ctx:
        const_pool = ctx.enter_context(tc.tile_pool(name="const", bufs=1))
        prep_pool = ctx.enter_context(tc.tile_pool(name="prep", bufs=1))
        in_pool = ctx.enter_context(tc.tile_pool(name="in", bufs=4))
        small_pool = ctx.enter_context(tc.tile_pool(name="small", bufs=4))
        batch_pool = ctx.enter_context(tc.tile_pool(name="batch", bufs=2))
        psum_pool = ctx.enter_context(tc.tile_pool(name="ps", bufs=2, space="PSUM"))

        # ---- constants ----
        ones_sb = const_pool.tile([P, 1], f32)
        nc.sync.dma_start(ones_sb[:], ones.ap())
        ident_sb = const_pool.tile([P, P], f32)
        nc.sync.dma_start(ident_sb[:], ident.ap())

        # ---- codebook -> normalized Gram table in HBM ----
        cb_sb = prep_pool.tile([P, 4, E], f32)
        cb_v = cb.ap().rearrange("(k p) e -> k p e", p=P)
        for k in range(4):
            nc.sync.dma_start(cb_sb[:, k, :], cb_v[k])

        sq = prep_pool.tile([P, E], f32)
        nrm2 = prep_pool.tile([P, 4], f32)
        for k in range(4):
            nc.scalar.activation(sq[:], cb_sb[:, k, :],
                                 mybir.ActivationFunctionType.Square,
                                 accum_out=nrm2[:, k:k + 1])
        nrm = prep_pool.tile([P, 4], f32)
        nc.scalar.sqrt(nrm[:], nrm2[:])
        rnorm = prep_pool.tile([P, 4], f32)
        nc.vector.reciprocal(rnorm[:], nrm[:])

        cn = prep_pool.tile([P, 4, E], f32)
        for k in range(4):
            nc.scalar.activation(cn[:, k, :], cb_sb[:, k, :],
                                 mybir.ActivationFunctionType.Copy,
                                 scale=rnorm[:, k:k + 1])

        # transpose Cn -> CnT[p, j, d]  (= Cn[d, j*128+p])
        cnT = prep_pool.tile([P, 2, D], f32)
        for k in range(4):
            for j in range(2):
                ps_t = psum_pool.tile([P, P], f32)
                nc.tensor.transpose(ps_t[:], cn[:, k, j * P:(j + 1) * P],
                                    ident_sb[:])
                nc.scalar.copy(cnT[:, j, k * P:(k + 1) * P], ps_t[:])

        # G = CnT.T @ CnT, in 4 partition chunks of 128 rows
        g_row_sb = prep_pool.tile([P, 4, D], f32)
        g_v = g_hbm.ap().rearrange("(r c) -> r c", c=D)
        for m in range(4):
            ps_g = psum_pool.tile([P, D], f32)
            for j in range(2):
                nc.tensor.matmul(ps_g[:], lhsT=cnT[:, j, m * P:(m + 1) * P],
                                 rhs=cnT[:, j, :], start=(j == 0), stop=(j == 1))
            nc.scalar.copy(g_row_sb[:, m, :], ps_g[:])
            nc.sync.dma_start(g_v[m * P:(m + 1) * P, :],
                              g_row_sb[:, m, :])
        zpad = prep_pool.tile([1, D], f32)
        nc.vector.memset(zpad[:], 0.0)
        nc.sync.dma_start(g_v[4 * P:4 * P + 1, :], zpad[:])

        # ---- per-tile top-8 ----
        idx_all = prep_pool.tile([P, T, G8], u32)
        pose_v = pose.ap().rearrange("(t p) d -> t p d", p=P)
        for t in range(T):
            pt = in_pool.tile([P, D], f32)
            nc.sync.dma_start(pt[:], pose_v[t])
            mx = small_pool.tile([P, G8], f32)
            nc.vector.max(mx[:], pt[:])
            nc.vector.max_index(idx_all[:, t, :], mx[:], pt[:])

        # ---- pair indices chunked by 4 tiles so gathers start early ----
        CH = 4
        a_f = prep_pool.tile([P, TB, NPAIR], f32)
        b_f = prep_pool.tile([P, TB, NPAIR], f32)
        pidx_f = prep_pool.tile([P, TB, NPAIR], f32)
        pidx_i = prep_pool.tile([P, TB, NPAIR], i32)
        acc4 = prep_pool.tile([P, TB, NPAIR], f32)
        for c0 in range(0, T, CH):
            sl_t = slice(c0, c0 + CH)
            idxb = idx_all[:, sl_t, :]
            for g in range(G8 - 1):
                o, w = _PAIR_OFF[g], G8 - 1 - g
                nc.vector.tensor_copy(
                    a_f[:, sl_t, o:o + w],
                    idxb[:, :, g:g + 1].broadcast_to([P, CH, w]))
                nc.vector.tensor_copy(b_f[:, sl_t, o:o + w],
                                      idxb[:, :, g + 1:G8])
            nc.vector.scalar_tensor_tensor(
                pidx_f[:, sl_t, :], a_f[:, sl_t, :], float(D),
                b_f[:, sl_t, :],
                op0=mybir.AluOpType.mult, op1=mybir.AluOpType.add)
            nc.vector.tensor_copy(pidx_i[:, sl_t, :], pidx_f[:, sl_t, :])
            for t in range(c0, c0 + CH):
                for sl in range(NPAIR):
                    nc.gpsimd.indirect_dma_start(
                        out=acc4[:, t, sl:sl + 1],
                        out_offset=None,
                        in_=g_hbm.ap().rearrange("(a b) -> a b", b=1),
                        in_offset=IndirectOffsetOnAxis(
                            ap=pidx_i[:, t, sl:sl + 1], axis=0),
                    )

        # ---- reduce: over TB, over partitions, AllReduce, abs-sum ----
        red = prep_pool.tile([P, NPAIR], f32)
        nc.vector.tensor_reduce(red[:], acc4[:].transpose([0, 2, 1]),
                                axis=mybir.AxisListType.X,
                                op=mybir.AluOpType.add)
        ps_r = psum_pool.tile([1, NPAIR], f32)
        nc.tensor.matmul(ps_r[:], lhsT=ones_sb[:], rhs=red[:],
                         start=True, stop=True)
        part = prep_pool.tile([1, NPAIR], f32)
        nc.scalar.copy(part[:], ps_r[:])

        nc.gpsimd.dma_start(ar_in.ap(), part[:])
        nc.gpsimd.collective_compute(
            "AllReduce", mybir.AluOpType.add,
            replica_groups=[list(range(N_CORES))],
            ins=[ar_in.ap()], outs=[ar_out.ap()],
        )
        allr = prep_pool.tile([1, NPAIR], f32)
        nc.gpsimd.dma_start(allr[:], ar_out.ap())
        abs_t = prep_pool.tile([1, NPAIR], f32)
        lsum = prep_pool.tile([1, 1], f32)
        nc.scalar.activation(abs_t[:], allr[:],
                             mybir.ActivationFunctionType.Abs,
                             accum_out=lsum[:])
        lout = prep_pool.tile([1, 1], f32)
        nc.scalar.mul(lout[:], lsum[:], 2.0 / float(BN))
        nc.gpsimd.dma_start(loss.ap(), lout[:])
        if debug:
            nc.sync.dma_start(d_idx.ap(), idx_all[:].rearrange("p t g -> p (t g)"))
            nc.sync.dma_start(d_acc.ap(), acc4[:].rearrange("p t s -> p (t s)"))
            nc.sync.dma_start(d_red.ap(), red[:])
            nc.sync.dma_start(d_part.ap(), part[:])
            nc.sync.dma_start(d_allr.ap(), allr[:])
            nc.sync.dma_start(d_g.ap(), g_row_sb[:].rearrange("p m d -> p (m d)"))
            nc.sync.dma_start(d_gath.ap(), gath4[:, :, :, 0].rearrange("p t s -> p (t s)"))
            nc.sync.dma_start(d_pidx.ap(), pidx_i[:].rearrange("p t s -> p (t s)"))

    nc.compile()
    return nc


_NC_CACHE = None


def _get_nc():
    global _NC_CACHE
    if _NC_CACHE is None:
        _NC_CACHE = build_nc()
    return _NC_CACHE


def make_in_maps(pose_code: np.ndarray, codebook: np.ndarray):
    flat = np.ascontiguousarray(
        pose_code.reshape(BN, D).astype(np.float32, copy=False))
    cbf = np.ascontiguousarray(codebook.astype(np.float32, copy=False))
    ident = np.eye(P, dtype=np.float32)
    ones = np.ones((P, 1), np.float32)
    in_maps = []
    for c in range(N_CORES):
        in_maps.append({
            "pose": flat[c * BN_PER_CORE:(c + 1) * BN_PER_CORE],
            "codebook": cbf,
            "ident": ident,
            "ones": ones,
        })
    return in_maps


def kernel(pose_code: np.ndarray, codebook: np.ndarray) -> np.ndarray:
    nc = _get_nc()
    in_maps = make_in_maps(pose_code, codebook)
    res = run_bass_kernel_spmd(nc, in_maps, core_ids=list(range(N_CORES)))
    out = np.asarray(res.results[0]["loss"], dtype=np.float32)
    return out.reshape(()).astype(np.float32)



# revision 9
# speedup vs baseline: 1.5473x; 1.5473x over previous
"""DisentangleLossBatch Trainium2 kernel (8 NeuronCores, data-parallel).

Math: loss = sum|mean_b(G[idx_g(b), idx_h(b)]) - I| over the 8x8 top-k
Gram matrix, where G = Cn @ Cn.T is the normalized-codebook Gram ([512,512])
and idx = top-8 indices of each token's 512 pose logits.

Strategy (v2, gpsimd ap_gather):
  * inner[b,g,h] = G[i_g, i_h], so per token only the 28 (g<h) Gram entries
    are needed; diagonal is 1 and the matrix is symmetric:
    loss = (2/BN) * sum_{g<h} |sum_b G[i_g(b), i_h(b)]|.
  * software-DGE indirect DMA moves at most 128 offsets per instruction
    (~1us fixed each), so gathering via DMA costs ~1.2ms.  Instead the
    Q7 DSP `ap_gather` gathers from SBUF: the G table is split 16-way
    column-wise inside each 16-partition group (gtab[p,x] =
    G.flat[(p%16)*16384 + x], 64KB/partition), and each (token, pair)
    becomes one gather column with shared index idx_lo = pidx % 16384.
    The correct value sits in partition q = pidx >> 14 of the group; a
    second ap_gather from a [128,16] one-hot SEL table produces the
    selection mask, and a DVE multiply + XY-reduce collapses
    (partitions x tokens) into per-pair sums.
  * pair index pidx = (i_g << 9) | i_h built with integer STTs.

Per core (4096 tokens): load pose tiles [128,512]; max8 + max_index;
per 8-tile chunk build pidx/idx_lo/q; per 4-tile slab ap_gather value +
mask, DVE mask-multiply and reduce to [128,28]; partition-reduce via
ones-matmul; AllReduce [1,28]; loss = (2/BN) * sum|entries|.
"""
import sys
import numpy as np

for _p in ("/opt/trn_rl_repo",):
    if _p not in sys.path:
        sys.path.insert(0, _p)

from contextlib import ExitStack

import concourse.bass as bass
import concourse.bacc as bacc
import concourse.tile as tile
import concourse.mybir as mybir
from concourse.bass_utils import run_bass_kernel_spmd

P = 128
N_CORES = 8
B, N, D, E = 32, 1024, 512, 256
G8 = 8
BN = B * N                       # 32768 tokens
BN_PER_CORE = BN // N_CORES      # 4096
T = BN_PER_CORE // P             # 32 tiles per core
NPAIR = (G8 * (G8 - 1)) // 2     # 28 strictly-upper pairs
CH = 8                           # tiles per pair-build chunk
GT = 4                           # tiles per ap_gather slab
NCOL = GT * 16 * NPAIR           # gather columns per slab (1792)
NSLAB = T // GT                  # 8 slabs
NE = D * D // 16                 # 16384 table elems per partition
f32 = mybir.dt.float32
i32 = mybir.dt.int32
u32 = mybir.dt.uint32
u16 = mybir.dt.uint16
i16 = mybir.dt.int16

_PAIR_OFF = []
_off = 0
for _g in range(G8 - 1):
    _PAIR_OFF.append(_off)
    _off += G8 - 1 - _g
assert _off == NPAIR


def build_nc():
    nc = bacc.Bacc("TRN2", target_bir_lowering=False, debug=False,
                   num_devices=N_CORES)
    pose = nc.dram_tensor("pose", [BN_PER_CORE, D], f32, kind="ExternalInput")
    cb = nc.dram_tensor("codebook", [D, E], f32, kind="ExternalInput")
    ident = nc.dram_tensor("ident", [P, P], f32, kind="ExternalInput")
    ones = nc.dram_tensor("ones", [P, 1], f32, kind="ExternalInput")
    sel_in = nc.dram_tensor("sel", [P, 16], f32, kind="ExternalInput")
    loss = nc.dram_tensor("loss", [1, 1], f32, kind="ExternalOutput")
    g_hbm = nc.dram_tensor("g_scratch", [D * D], f32)
    ar_in = nc.dram_tensor("ar_in", [1, NPAIR], f32)
    ar_out = nc.dram_tensor("ar_out", [1, NPAIR], f32, addr_space="Shared")

    with tile.TileContext(nc) as tc, ExitStack() as ctx:
        const_pool = ctx.enter_context(tc.tile_pool(name="const", bufs=1))
        prep_pool = ctx.enter_context(tc.tile_pool(name="prep", bufs=1))
        in_pool = ctx.enter_context(tc.tile_pool(name="in", bufs=6))
        small_pool = ctx.enter_context(tc.tile_pool(name="small", bufs=4))
        slab_pool = ctx.enter_context(tc.tile_pool(name="slab", bufs=2))
        psum_pool = ctx.enter_context(tc.tile_pool(name="ps", bufs=2, space="PSUM"))

        # ---- constants ----
        ones_sb = const_pool.tile([P, 1], f32)
        nc.sync.dma_start(ones_sb[:], ones.ap())
        ident_sb = const_pool.tile([P, P], f32)
        nc.sync.dma_start(ident_sb[:], ident.ap())
        sel_sb = const_pool.tile([P, 16, 1], f32)
        nc.sync.dma_start(sel_sb[:].rearrange("p a b -> p (a b)"), sel_in.ap())

        # ---- codebook -> normalized Gram table (HBM roundtrip) ----
        cb_sb = prep_pool.tile([P, 4, E], f32)
        cb_v = cb.ap().rearrange("(k p) e -> k p e", p=P)
        for k in range(4):
            nc.sync.dma_start(cb_sb[:, k, :], cb_v[k])

        sq = prep_pool.tile([P, E], f32)
        nrm2 = prep_pool.tile([P, 4], f32)
        for k in range(4):
            nc.scalar.activation(sq[:], cb_sb[:, k, :],
                                 mybir.ActivationFunctionType.Square,
                                 accum_out=nrm2[:, k:k + 1])
        nrm = prep_pool.tile([P, 4], f32)
        nc.scalar.sqrt(nrm[:], nrm2[:])
        rnorm = prep_pool.tile([P, 4], f32)
        nc.vector.reciprocal(rnorm[:], nrm[:])

        cn = prep_pool.tile([P, 4, E], f32)
        for k in range(4):
            nc.scalar.activation(cn[:, k, :], cb_sb[:, k, :],
                                 mybir.ActivationFunctionType.Copy,
                                 scale=rnorm[:, k:k + 1])

        cnT = prep_pool.tile([P, 2, D], f32)
        for k in range(4):
            for j in range(2):
                ps_t = psum_pool.tile([P, P], f32)
                nc.tensor.transpose(ps_t[:], cn[:, k, j * P:(j + 1) * P],
                                    ident_sb[:])
                nc.scalar.copy(cnT[:, j, k * P:(k + 1) * P], ps_t[:])

        g_row_sb = prep_pool.tile([P, 4, D], f32)
        g_v = g_hbm.ap().rearrange("(r c) -> r c", c=D)
        for m in range(4):
            ps_g = psum_pool.tile([P, D], f32)
            for j in range(2):
                nc.tensor.matmul(ps_g[:], lhsT=cnT[:, j, m * P:(m + 1) * P],
                                 rhs=cnT[:, j, :], start=(j == 0), stop=(j == 1))
            nc.scalar.copy(g_row_sb[:, m, :], ps_g[:])
            nc.sync.dma_start(g_v[m * P:(m + 1) * P, :],
                              g_row_sb[:, m, :])

        # gtab[p, x] = G.flat[(p%16)*16384 + x]; one DMA per 16-part group
        gtab = prep_pool.tile([P, NE, 1], f32)
        src = g_hbm.ap().rearrange("(q x) -> q x", x=NE)
        for g in range(8):
            nc.sync.dma_start(gtab[16 * g:16 * (g + 1), :, 0], src)

        # ---- top-8, pair indices, gathers, mask-reduce ----
        zero32 = const_pool.tile([P, 1, 1], u32)
        nc.vector.memset(zero32[:], 0)
        idx_all = prep_pool.tile([P, T, G8], u32)
        pidx = prep_pool.tile([P, T, NPAIR], u32)
        ilo = prep_pool.tile([P, T, NPAIR], u32)
        qhi = prep_pool.tile([P, T, NPAIR], u32)
        ilo16 = prep_pool.tile([P, T, NPAIR], i16)
        qhi16 = prep_pool.tile([P, T, NPAIR], i16)
        red = prep_pool.tile([P, NSLAB, NPAIR], f32)
        pose_v = pose.ap().rearrange("(t p) d -> t p d", p=P)

        def emit_slab(s):
            sl = slice(s * GT, (s + 1) * GT)
            gv = slab_pool.tile([P, NCOL, 1], f32)
            nc.gpsimd.ap_gather(
                gv[:], gtab[:], ilo16[:, sl, :].rearrange("p t s -> p (t s)"),
                channels=P, num_elems=NE, d=1, num_idxs=NCOL)
            gm = slab_pool.tile([P, NCOL, 1], f32)
            nc.gpsimd.ap_gather(
                gm[:], sel_sb[:], qhi16[:, sl, :].rearrange("p t s -> p (t s)"),
                channels=P, num_elems=16, d=1, num_idxs=NCOL)
            mv = slab_pool.tile([P, NCOL], f32)
            nc.vector.tensor_tensor(mv[:], gv[:, :, 0], gm[:, :, 0],
                                    op=mybir.AluOpType.mult)
            # columns are (t_local, s, r): reduce tokens (t_local, r), keep s
            nc.vector.tensor_reduce(
                red[:, s, :],
                mv[:].rearrange("p (t s r) -> p t s r", t=GT, s=NPAIR, r=16)
                     .transpose([0, 2, 1, 3]),
                axis=mybir.AxisListType.XY,
                op=mybir.AluOpType.add)

        for t in range(T):
            pt = in_pool.tile([P, D], f32)
            nc.sync.dma_start(pt[:], pose_v[t])
            mx = small_pool.tile([P, G8], f32)
            nc.vector.max(mx[:], pt[:])
            nc.vector.max_index(idx_all[:, t, :], mx[:], pt[:])
            if (t + 1) % CH == 0:
                sl = slice(t + 1 - CH, t + 1)
                idxb = idx_all[:, sl, :]
                for g in range(G8 - 1):
                    o, w = _PAIR_OFF[g], G8 - 1 - g
                    stt = nc.vector.scalar_tensor_tensor(
                        pidx[:, sl, o:o + w],
                        idxb[:, :, g:g + 1].broadcast_to([P, CH, w]),
                        9,
                        idxb[:, :, g + 1:G8],
                        op0=mybir.AluOpType.logical_shift_left,
                        op1=mybir.AluOpType.bitwise_or)
                    stt.ins.ins[1] = mybir.ImmediateValue(dtype=u32, value=9)
                zbc = zero32[:].broadcast_to([P, CH, NPAIR])
                ts1 = nc.vector.scalar_tensor_tensor(
                    ilo[:, sl, :], pidx[:, sl, :], 16383, zbc,
                    op0=mybir.AluOpType.bitwise_and,
                    op1=mybir.AluOpType.bitwise_or)
                ts1.ins.ins[1] = mybir.ImmediateValue(dtype=u32, value=16383)
                ts2 = nc.vector.scalar_tensor_tensor(
                    qhi[:, sl, :], pidx[:, sl, :], 14, zbc,
                    op0=mybir.AluOpType.logical_shift_right,
                    op1=mybir.AluOpType.bitwise_or)
                ts2.ins.ins[1] = mybir.ImmediateValue(dtype=u32, value=14)
                nc.vector.tensor_copy(ilo16[:, sl, :], ilo[:, sl, :])
                nc.vector.tensor_copy(qhi16[:, sl, :], qhi[:, sl, :])
                emit_slab((t + 1) // GT - 2)
                emit_slab((t + 1) // GT - 1)

        # ---- final reduce: slabs, partitions, AllReduce, abs-sum ----
        redf = prep_pool.tile([P, NPAIR], f32)
        nc.vector.tensor_reduce(redf[:], red[:].transpose([0, 2, 1]),
                                axis=mybir.AxisListType.X,
                                op=mybir.AluOpType.add)
        ps_r = psum_pool.tile([1, NPAIR], f32)
        nc.tensor.matmul(ps_r[:], lhsT=ones_sb[:], rhs=redf[:],
                         start=True, stop=True)
        part = prep_pool.tile([1, NPAIR], f32)
        nc.scalar.copy(part[:], ps_r[:])

        nc.gpsimd.dma_start(ar_in.ap(), part[:])
        nc.gpsimd.collective_compute(
            "AllReduce", mybir.AluOpType.add,
            replica_groups=[list(range(N_CORES))],
            ins=[ar_in.ap()], outs=[ar_out.ap()],
        )
        allr = prep_pool.tile([1, NPAIR], f32)
        nc.gpsimd.dma_start(allr[:], ar_out.ap())
        abs_t = prep_pool.tile([1, NPAIR], f32)
        lsum = prep_pool.tile([1, 1], f32)
        nc.scalar.activation(abs_t[:], allr[:],
                             mybir.ActivationFunctionType.Abs,
                             accum_out=lsum[:])
        lout = prep_pool.tile([1, 1], f32)
        nc.scalar.mul(lout[:], lsum[:], 2.0 / float(BN))
        nc.gpsimd.dma_start(loss.ap(), lout[:])

    nc.compile()
    return nc


_NC_CACHE = None


def _get_nc():
    global _NC_CACHE
    if _NC_CACHE is None:
        _NC_CACHE = build_nc()
    return _NC_CACHE


def make_in_maps(pose_code: np.ndarray, codebook: np.ndarray):
    flat = np.ascontiguousarray(
        pose_code.reshape(BN, D).astype(np.float32, copy=False))
    cbf = np.ascontiguousarray(codebook.astype(np.float32, copy=False))
    ident = np.eye(P, dtype=np.float32)
    ones = np.ones((P, 1), np.float32)
    sel = (np.arange(P)[:, None] % 16 == np.arange(16)[None, :]).astype(
        np.float32)
    in_maps = []
    for c in range(N_CORES):
        in_maps.append({
            "pose": flat[c * BN_PER_CORE:(c + 1) * BN_PER_CORE],
            "codebook": cbf,
            "ident": ident,
            "ones": ones,
            "sel": sel,
        })
    return in_maps


def kernel(pose_code: np.ndarray, codebook: np.ndarray) -> np.ndarray:
    nc = _get_nc()
    in_maps = make_in_maps(pose_code, codebook)
    res = run_bass_kernel_spmd(nc, in_maps, core_ids=list(range(N_CORES)))
    out = np.asarray(res.results[0]["loss"], dtype=np.float32)
    return out.reshape(()).astype(np.float32)


# revision 16
# speedup vs baseline: 1.5760x; 1.0186x over previous
"""DisentangleLossBatch Trainium2 kernel (8 NeuronCores, data-parallel).

Math: loss = sum|mean_b(G[idx_g(b), idx_h(b)]) - I| over the 8x8 top-k
Gram matrix, where G = Cn @ Cn.T is the normalized-codebook Gram ([512,512])
and idx = top-8 indices of each token's 512 pose logits.

Strategy (v2.3, dual concurrent gather engines):
  * inner[b,g,h] = G[i_g, i_h]; diagonal is 1 and the matrix symmetric:
    loss = (2/BN) * sum_{g<h} |sum_b G[i_g(b), i_h(b)]|, so each token
    needs 28 gathered Gram entries (pidx = (i_g<<9)|i_h).
  * Two gather resources inside the Pool complex run CONCURRENTLY
    (verified on HW):
    - Q7 DSPs via `ap_gather` (~25-32ns/column): G split 16-way
      column-wise per 16-partition group (gtab[p,x] =
      G.flat[(p%16)*16384 + x]); each (token,pair) is one column with
      shared index idx_lo = pidx & 16383.  The right value sits in
      partition q = pidx>>14; q is broadcast to the group via an HBM
      round-trip (stride-0 partition DMA) and one fused DVE STT
      computes (q == p%16) * value.
    - Pool sequencer via swdge indirect DMA (~1.1us per 128-offset
      [128,1] column gather), issued between DSP launches so both
      gather engines run at once.
  * top-8 via DVE max8/max_index in f32 (order must match jax top_k).

Per core (4096 tokens = 32 tiles): per 8-tile chunk, 6 tiles go to DSP
slabs (2 tiles each) and 2 to swdge (last chunk: 4/4); partial sums
reduce to [128,28]; ones-matmul partition reduce; AllReduce [1,28];
abs-sum; scale by 2/BN.
"""
import sys
import numpy as np

for _p in ("/opt/trn_rl_repo",):
    if _p not in sys.path:
        sys.path.insert(0, _p)

from contextlib import ExitStack

import concourse.bass as bass
import concourse.bacc as bacc
import concourse.tile as tile
import concourse.mybir as mybir
from concourse.bass import IndirectOffsetOnAxis
from concourse.bass_utils import run_bass_kernel_spmd

P = 128
N_CORES = 8
B, N, D, E = 32, 1024, 512, 256
G8 = 8
BN = B * N                       # 32768 tokens
BN_PER_CORE = BN // N_CORES      # 4096
T = BN_PER_CORE // P             # 32 tiles per core
NPAIR = (G8 * (G8 - 1)) // 2     # 28 strictly-upper pairs
CH = 8                           # tiles per pair-build chunk
GT = 2                           # tiles per DSP slab
NCOL = GT * 16 * NPAIR           # gather columns per slab (896)
NE = D * D // 16                 # 16384 table elems per partition
f32 = mybir.dt.float32
i32 = mybir.dt.int32
u32 = mybir.dt.uint32
i16 = mybir.dt.int16

# per chunk: first DSP_PER_CHUNK tiles -> DSP slabs, rest -> swdge
DSP_PER_CHUNK = [6, 6, 6, 4]
SLABS = []                       # (slab_id, first_tile)
SW_TILES = []
for _c in range(T // CH):
    _c0 = _c * CH
    _nd = DSP_PER_CHUNK[_c]
    for _k in range(0, _nd, GT):
        SLABS.append(_c0 + _k)
    SW_TILES.extend(range(_c0 + _nd, _c0 + CH))
NSLABS = len(SLABS)              # 11
NSW = len(SW_TILES)              # 10

_PAIR_OFF = []
_off = 0
for _g in range(G8 - 1):
    _PAIR_OFF.append(_off)
    _off += G8 - 1 - _g
assert _off == NPAIR


def build_nc():
    nc = bacc.Bacc("TRN2", target_bir_lowering=False, debug=False,
                   num_devices=N_CORES)
    pose = nc.dram_tensor("pose", [BN_PER_CORE, D], f32, kind="ExternalInput")
    cb = nc.dram_tensor("codebook", [D, E], f32, kind="ExternalInput")
    ident = nc.dram_tensor("ident", [P, P], f32, kind="ExternalInput")
    ones = nc.dram_tensor("ones", [P, 1], f32, kind="ExternalInput")
    pmod_in = nc.dram_tensor("pmod", [P, 1], f32, kind="ExternalInput")
    loss = nc.dram_tensor("loss", [1, 1], f32, kind="ExternalOutput")
    g_hbm = nc.dram_tensor("g_scratch", [D * D], f32)
    q_hbm = nc.dram_tensor("q_scratch", [8 * NSLABS * NCOL], f32)
    ar_in = nc.dram_tensor("ar_in", [1, NPAIR], f32)
    ar_out = nc.dram_tensor("ar_out", [1, NPAIR], f32, addr_space="Shared")

    with tile.TileContext(nc) as tc, ExitStack() as ctx:
        const_pool = ctx.enter_context(tc.tile_pool(name="const", bufs=1))
        prep_pool = ctx.enter_context(tc.tile_pool(name="prep", bufs=1))
        in_pool = ctx.enter_context(tc.tile_pool(name="in", bufs=6))
        small_pool = ctx.enter_context(tc.tile_pool(name="small", bufs=4))
        post_pool = ctx.enter_context(tc.tile_pool(name="post", bufs=2))
        psum_pool = ctx.enter_context(tc.tile_pool(name="ps", bufs=2, space="PSUM"))

        # ---- constants ----
        ones_sb = const_pool.tile([P, 1], f32)
        nc.sync.dma_start(ones_sb[:], ones.ap())
        ident_sb = const_pool.tile([P, P], f32)
        nc.sync.dma_start(ident_sb[:], ident.ap())
        pmod_sb = const_pool.tile([P, 1], f32)
        nc.sync.dma_start(pmod_sb[:], pmod_in.ap())

        # ---- codebook -> normalized Gram table (HBM roundtrip) ----
        cb_sb = prep_pool.tile([P, 4, E], f32)
        cb_v = cb.ap().rearrange("(k p) e -> k p e", p=P)
        for k in range(4):
            nc.sync.dma_start(cb_sb[:, k, :], cb_v[k])

        sq = prep_pool.tile([P, E], f32)
        nrm2 = prep_pool.tile([P, 4], f32)
        for k in range(4):
            nc.scalar.activation(sq[:], cb_sb[:, k, :],
                                 mybir.ActivationFunctionType.Square,
                                 accum_out=nrm2[:, k:k + 1])
        nrm = prep_pool.tile([P, 4], f32)
        nc.scalar.sqrt(nrm[:], nrm2[:])
        rnorm = prep_pool.tile([P, 4], f32)
        nc.vector.reciprocal(rnorm[:], nrm[:])

        cn = prep_pool.tile([P, 4, E], f32)
        for k in range(4):
            nc.scalar.activation(cn[:, k, :], cb_sb[:, k, :],
                                 mybir.ActivationFunctionType.Copy,
                                 scale=rnorm[:, k:k + 1])

        cnT = prep_pool.tile([P, 2, D], f32)
        for k in range(4):
            for j in range(2):
                ps_t = psum_pool.tile([P, P], f32)
                nc.tensor.transpose(ps_t[:], cn[:, k, j * P:(j + 1) * P],
                                    ident_sb[:])
                nc.scalar.copy(cnT[:, j, k * P:(k + 1) * P], ps_t[:])

        g_row_sb = prep_pool.tile([P, 4, D], f32)
        g_v = g_hbm.ap().rearrange("(r c) -> r c", c=D)
        for m in range(4):
            ps_g = psum_pool.tile([P, D], f32)
            for j in range(2):
                nc.tensor.matmul(ps_g[:], lhsT=cnT[:, j, m * P:(m + 1) * P],
                                 rhs=cnT[:, j, :], start=(j == 0), stop=(j == 1))
            nc.scalar.copy(g_row_sb[:, m, :], ps_g[:])
            nc.sync.dma_start(g_v[m * P:(m + 1) * P, :],
                              g_row_sb[:, m, :])

        # gtab[p, x] = G.flat[(p%16)*16384 + x]; one DMA per 16-part group
        gtab = prep_pool.tile([P, NE, 1], f32)
        gsrc = g_hbm.ap().rearrange("(q x) -> q x", x=NE)
        for g in range(8):
            nc.sync.dma_start(gtab[16 * g:16 * (g + 1), :, 0], gsrc)

        # ---- top-8, pair indices, dual-engine gathers ----
        zero32 = const_pool.tile([P, 1, 1], u32)
        nc.vector.memset(zero32[:], 0)
        idx_all = prep_pool.tile([P, T, G8], u32)
        pidx = prep_pool.tile([P, T, NPAIR], u32)
        ilo = prep_pool.tile([P, T, NPAIR], u32)
        ilo16 = prep_pool.tile([P, T, NPAIR], i16)
        qhf = prep_pool.tile([P, T, NPAIR], f32)
        gv_all = prep_pool.tile([P, NSLABS, NCOL], f32)
        red = prep_pool.tile([P, NSLABS + 1, NPAIR], f32)
        acc_sw = prep_pool.tile([P, NSW, NPAIR], f32)
        pose_v = pose.ap().rearrange("(t p) d -> t p d", p=P)

        # q_hbm viewed [group, slab*(t_local s)*r]
        q_rows = q_hbm.ap().rearrange("(g n) -> g n", n=NSLABS * NCOL)

        def emit_q_write(k0, t0, nd):
            # chunk write: slabs k0.. for tiles [t0, t0+nd);
            # flat n' = (t-t0)*NPAIR+s is contiguous across the chunk
            nn = nd * NPAIR
            for g in range(8):
                dst = q_rows[g, k0 * NCOL:k0 * NCOL + nn * 16].rearrange(
                    "(n r) -> r n", r=16)
                nc.sync.dma_start(
                    dst,
                    qhf[16 * g:16 * (g + 1), t0:t0 + nd, :]
                    .rearrange("p t s -> p (t s)"))

        def emit_launch(k, t0):
            out = gv_all[:, k, :].rearrange("p (n o) -> p n o", o=1)
            nc.gpsimd.ap_gather(
                out, gtab[:],
                ilo16[:, t0:t0 + GT, :].rearrange("p t s -> p (t s)"),
                channels=P, num_elems=NE, d=1, num_idxs=NCOL)

        def emit_post(k):
            mq = post_pool.tile([P, NCOL], f32)
            for g in range(8):
                nc.scalar.dma_start(
                    mq[16 * g:16 * (g + 1), :],
                    q_rows[g, k * NCOL:(k + 1) * NCOL]
                    .rearrange("(a n) -> a n", a=1).broadcast_to([16, NCOL]))
            mv = post_pool.tile([P, NCOL], f32)
            nc.vector.scalar_tensor_tensor(
                mv[:], mq[:], pmod_sb[:, 0:1], gv_all[:, k, :],
                op0=mybir.AluOpType.is_equal,
                op1=mybir.AluOpType.mult)
            nc.vector.tensor_reduce(
                red[:, k, :],
                mv[:].rearrange("p (t s r) -> p t s r", t=GT, s=NPAIR, r=16)
                     .transpose([0, 2, 1, 3]),
                axis=mybir.AxisListType.XY,
                op=mybir.AluOpType.add)

        def emit_swdge(t):
            slot = SW_TILES.index(t)
            for s in range(NPAIR):
                nc.gpsimd.indirect_dma_start(
                    out=acc_sw[:, slot, s:s + 1],
                    out_offset=None,
                    in_=g_hbm.ap().rearrange("(a b) -> a b", b=1),
                    in_offset=IndirectOffsetOnAxis(
                        ap=pidx[:, t, s:s + 1].bitcast(i32), axis=0),
                )

        for c in range(T // CH):
            c0 = c * CH
            for t in range(c0, c0 + CH):
                pt = in_pool.tile([P, D], f32)
                nc.sync.dma_start(pt[:], pose_v[t])
                mx = small_pool.tile([P, G8], f32)
                nc.vector.max(mx[:], pt[:])
                nc.vector.max_index(idx_all[:, t, :], mx[:], pt[:])
            sl = slice(c0, c0 + CH)
            idxb = idx_all[:, sl, :]
            for g in range(G8 - 1):
                o, w = _PAIR_OFF[g], G8 - 1 - g
                stt = nc.vector.scalar_tensor_tensor(
                    pidx[:, sl, o:o + w],
                    idxb[:, :, g:g + 1].broadcast_to([P, CH, w]),
                    9,
                    idxb[:, :, g + 1:G8],
                    op0=mybir.AluOpType.logical_shift_left,
                    op1=mybir.AluOpType.bitwise_or)
                stt.ins.ins[1] = mybir.ImmediateValue(dtype=u32, value=9)
            nd = DSP_PER_CHUNK[c]
            if nd:
                dsl = slice(c0, c0 + nd)
                zbc = zero32[:].broadcast_to([P, nd, NPAIR])
                ts1 = nc.vector.scalar_tensor_tensor(
                    ilo[:, dsl, :], pidx[:, dsl, :], 16383, zbc,
                    op0=mybir.AluOpType.bitwise_and,
                    op1=mybir.AluOpType.bitwise_or)
                ts1.ins.ins[1] = mybir.ImmediateValue(dtype=u32, value=16383)
                qh32 = prep_pool.tile([P, nd, NPAIR], u32)
                ts2 = nc.vector.scalar_tensor_tensor(
                    qh32[:], pidx[:, dsl, :], 14, zbc,
                    op0=mybir.AluOpType.logical_shift_right,
                    op1=mybir.AluOpType.bitwise_or)
                ts2.ins.ins[1] = mybir.ImmediateValue(dtype=u32, value=14)
                nc.vector.tensor_copy(ilo16[:, dsl, :], ilo[:, dsl, :])
                nc.vector.tensor_copy(qhf[:, dsl, :], qh32[:])
            # interleave: [launch slab][swdge tile][launch slab]...
            slabs_c = [(SLABS.index(t0), t0) for t0 in SLABS
                       if c0 <= t0 < c0 + CH]
            sws_c = [t for t in SW_TILES if c0 <= t < c0 + CH]
            if nd:
                emit_q_write(slabs_c[0][0], c0, nd)
            li, si = 0, 0
            while li < len(slabs_c) or si < len(sws_c):
                if li < len(slabs_c):
                    emit_launch(*slabs_c[li]); li += 1
                if si < len(sws_c):
                    emit_swdge(sws_c[si]); si += 1

        for k in range(NSLABS):
            emit_post(k)

        # swdge partial reduce into red[:, NSLABS, :]
        nc.vector.tensor_reduce(red[:, NSLABS, :],
                                acc_sw[:].transpose([0, 2, 1]),
                                axis=mybir.AxisListType.X,
                                op=mybir.AluOpType.add)

        # ---- final reduce: slabs, partitions, AllReduce, abs-sum ----
        redf = prep_pool.tile([P, NPAIR], f32)
        nc.vector.tensor_reduce(redf[:], red[:].transpose([0, 2, 1]),
                                axis=mybir.AxisListType.X,
                                op=mybir.AluOpType.add)
        ps_r = psum_pool.tile([1, NPAIR], f32)
        nc.tensor.matmul(ps_r[:], lhsT=ones_sb[:], rhs=redf[:],
                         start=True, stop=True)
        part = prep_pool.tile([1, NPAIR], f32)
        nc.scalar.copy(part[:], ps_r[:])

        nc.gpsimd.dma_start(ar_in.ap(), part[:])
        nc.gpsimd.collective_compute(
            "AllReduce", mybir.AluOpType.add,
            replica_groups=[list(range(N_CORES))],
            ins=[ar_in.ap()], outs=[ar_out.ap()],
        )
        allr = prep_pool.tile([1, NPAIR], f32)
        nc.gpsimd.dma_start(allr[:], ar_out.ap())
        abs_t = prep_pool.tile([1, NPAIR], f32)
        lsum = prep_pool.tile([1, 1], f32)
        nc.scalar.activation(abs_t[:], allr[:],
                             mybir.ActivationFunctionType.Abs,
                             accum_out=lsum[:])
        lout = prep_pool.tile([1, 1], f32)
        nc.scalar.mul(lout[:], lsum[:], 2.0 / float(BN))
        nc.gpsimd.dma_start(loss.ap(), lout[:])

    nc.compile()
    return nc


_NC_CACHE = None


def _get_nc():
    global _NC_CACHE
    if _NC_CACHE is None:
        _NC_CACHE = build_nc()
    return _NC_CACHE


def make_in_maps(pose_code: np.ndarray, codebook: np.ndarray):
    flat = np.ascontiguousarray(
        pose_code.reshape(BN, D).astype(np.float32, copy=False))
    cbf = np.ascontiguousarray(codebook.astype(np.float32, copy=False))
    ident = np.eye(P, dtype=np.float32)
    ones = np.ones((P, 1), np.float32)
    pmod = (np.arange(P) % 16).astype(np.float32).reshape(P, 1)
    in_maps = []
    for c in range(N_CORES):
        in_maps.append({
            "pose": flat[c * BN_PER_CORE:(c + 1) * BN_PER_CORE],
            "codebook": cbf,
            "ident": ident,
            "ones": ones,
            "pmod": pmod,
        })
    return in_maps


def kernel(pose_code: np.ndarray, codebook: np.ndarray) -> np.ndarray:
    nc = _get_nc()
    in_maps = make_in_maps(pose_code, codebook)
    res = run_bass_kernel_spmd(nc, in_maps, core_ids=list(range(N_CORES)))
    out = np.asarray(res.results[0]["loss"], dtype=np.float32)
    return out.reshape(()).astype(np.float32)
